# revision 1
# baseline (speedup 1.0000x reference)
"""BottleneckAttention3D kernel for 8 Trainium2 NeuronCores.

Reference computation (per batch b):
    h = GroupNorm(x)                      # [C, N], C=128, N=4096, 8 groups
    q = wq @ h + bq ; k = wk @ h + bk ; v = wv @ h + bv
    attn = softmax(q.T k / sqrt(C))       # [N, N]
    out = v attn.T ; y = x + wp @ out + bp

Sharding: 8 cores = 2 batches x 4 query blocks of NQ=1024 tokens. Each core
computes K/V for its whole batch and Q for its query block, then runs a
flash-attention-style loop over 32 key blocks of 128 tokens; the N^2 score
matrix lives only in PSUM/SBUF.

Host preprocessing (cheap, 0.2% of FLOPs): groupnorm statistics and the
affine fold into the QKV weights (W' = W diag(s), b' = W t + b), plus
weight transposes and fp16 casts of x.

Device-side structure per core:
  * K = Wk' x, V = (Wv' x)^T, q^T = Wq'' x_s + bq'' as fp16 tiles. The K
    bias is dropped entirely: softmax is invariant to per-query shifts.
    The V bias reduces to an additive constant (rows of attn sum to 1),
    folded into the projection bias on host.
  * Main loop (software-pipelined): scoresT block = K-block^T Q (fp16
    matmuls, f32 PSUM) -> exp on ACT with the q-bias score term folded
    into the per-partition activation bias (no max subtraction; scores
    are O(6)) -> fp16 E tile -> attention*V accumulated in PSUM. The
    softmax denominator sum(E) accumulates on the Vector engine (gpsimd
    shares the DVE SBUF port, so it cannot help) except the last 4
    blocks, which go to PE ones-matmuls accumulating in PSUM so no add
    chain trails the loop.
  * 1/d via a K=1 ones broadcast matmul + reciprocal_approx_fast (51 ULP,
    plenty for a softmax denominator), then projection + residual. All
    ACT functions stay inside one table set (single table load).
"""

import sys

sys.path.insert(0, "/opt/trn_rl_repo")

import numpy as np

B = 2
C = 128
N = 4096  # 16*16*16 tokens
NQ = N // 4  # query block per core (1024)
GROUPS = 8
EPS = 1e-5
XCH = 1024
NX = N // XCH  # 4
MB = N // 128  # 32 key blocks
_CACHE = {}


def _build():
    import concourse.bacc as bacc
    import concourse.mybir as mybir
    import concourse.tile as tile

    F32 = mybir.dt.float32
    F32R = mybir.dt.float32r
    F16 = mybir.dt.float16
    Exp = mybir.ActivationFunctionType.Exp
    Copy = mybir.ActivationFunctionType.Copy

    nc = bacc.Bacc("TRN2", target_bir_lowering=False, debug=False)

    # ---- DRAM I/O ----
    xh_d = nc.dram_tensor("xh", [C, N], F16, kind="ExternalInput")
    xsh_d = nc.dram_tensor("xsh", [C, NQ], F16, kind="ExternalInput")
    xs_d = nc.dram_tensor("xs", [C, NQ], F32, kind="ExternalInput")
    wf_d = nc.dram_tensor("wf", [C, 3 * C], F16, kind="ExternalInput")  # wq|wk|wv
    wpt_d = nc.dram_tensor("wpt", [C, C], F32R, kind="ExternalInput")
    fcols_d = nc.dram_tensor("fcols", [C, 1 + MB], F32, kind="ExternalInput")
    y_d = nc.dram_tensor("y", [C, NQ], F32, kind="ExternalOutput")

    with tile.TileContext(nc) as tc:
        with (
            tc.tile_pool(name="cst", bufs=1) as cst,
            tc.tile_pool(name="xp", bufs=1) as xp,
            tc.tile_pool(name="ep", bufs=10) as ep,
            tc.tile_pool(name="psm", bufs=2, space="PSUM") as psm,
            tc.tile_pool(name="pso", bufs=1, space="PSUM") as pso,
        ):
            # dummy ACT op: load the ln+exp table set at t=0
            DUM = cst.tile([1, 1], F32, tag="dum")
            nc.vector.memset(DUM, 1.0)
            DUM2 = cst.tile([1, 1], F32, tag="dum2")
            nc.scalar.activation(DUM2, DUM, Exp)

            # ---- input loads ----
            XH = []
            for j in range(NX):
                xt = xp.tile([C, XCH], F16, tag=f"x{j}", name=f"x{j}")
                nc.sync.dma_start(xt, xh_d[:, j * XCH : (j + 1) * XCH])
                XH.append(xt)
            XSH = cst.tile([C, NQ], F16, tag="xsh")
            nc.sync.dma_start(XSH, xsh_d[:, :])
            XS = cst.tile([C, NQ], F32, tag="xs")
            nc.sync.dma_start(XS, xs_d[:, :])
            WF = cst.tile([C, 3 * C], F16, tag="wf")
            nc.gpsimd.dma_start(WF, wf_d[:, :])
            WPT = cst.tile([C, C], F32R, tag="wpt")
            nc.gpsimd.dma_start(WPT, wpt_d[:, :])
            FCOLS = cst.tile([C, 1 + MB], F32, tag="fcols")
            nc.gpsimd.dma_start(FCOLS, fcols_d[:, :])
            WQF = WF[:, 0 * C : 1 * C]
            WKF = WF[:, 1 * C : 2 * C]
            WVF = WF[:, 2 * C : 3 * C]
            FB = FCOLS[:, 0:1]
            BT = FCOLS[:, 1:]
            # ones vectors built on device (f16 memset; f32r via ACT copy)
            ONH = cst.tile([C, 1], F16, tag="onh")
            nc.vector.memset(ONH, 1.0)
            ONF = cst.tile([C, 2], F32, tag="onf")
            nc.vector.memset(ONF, 1.0)
            ONC = cst.tile([C, 1], F32R, tag="onc")
            nc.scalar.activation(ONC, ONF[:, 0:1], Copy)
            ONRF = cst.tile([1, C], F32, tag="onrf")
            nc.vector.memset(ONRF, 1.0)
            ONR = cst.tile([1, C], F32R, tag="onr")
            nc.scalar.activation(ONR, ONRF, Copy)

            # ---- Q then K (fp16; k-bias dropped: softmax shift-invariant) ----
            PQ = psm.tile([C, NQ], F32, tag="psq", name="pq")
            for h in range(2):
                sl = slice(h * 512, (h + 1) * 512)
                nc.tensor.matmul(PQ[:, sl], WQF, XSH[:, sl], start=True, stop=True)
            QT = cst.tile([C, NQ], F16, tag="qt")
            nc.scalar.activation(QT, PQ, Copy)
            K = []
            for j2 in range(2 * NX):
                pk = psm.tile([C, 512], F32, tag="ps", name=f"pk{j2}")
                nc.tensor.matmul(
                    pk, WKF, XH[j2 // 2][:, (j2 % 2) * 512 : (j2 % 2 + 1) * 512],
                    start=True, stop=True,
                )
                kt = xp.tile([C, 512], F16, tag=f"k{j2}", name=f"k{j2}")
                nc.scalar.activation(kt, pk, Copy)
                K.append(kt)
            V = [None] * (2 * NX)

            # ---- main attention loop ----
            PO = pso.tile([C, NQ], F32, tag="po")
            ACCF = cst.tile([C, NQ], F32R, tag="accf")
            EL = [None] * MB
            PD = [None, None]

            def av(i):
                g, u = i // 4, i % 4
                for h in range(2):
                    sl = slice(h * 512, (h + 1) * 512)
                    nc.tensor.matmul(
                        PO[:, sl], V[g][:, u, :], EL[i][:, sl],
                        start=(i == 0), stop=(i == MB - 1),
                    )

            def make_v(g):
                pv = psm.tile([C, 4, 128], F32, tag="ps", name=f"pv{g}", bufs=2)
                for w in range(4):
                    m0 = (g % 2) * 512 + w * 128
                    nc.tensor.matmul(
                        pv[:, w, :],
                        XH[g // 2][:, m0 : m0 + 128],
                        WVF,
                        start=True,
                        stop=True,
                    )
                vt = xp.tile([C, 4, 128], F16, tag=f"v{g}", name=f"v{g}")
                nc.vector.tensor_copy(vt, pv)
                V[g] = vt

            make_v(0)
            for i in range(MB):
                g, u = i // 4, i % 4
                if u == 2 and g + 1 < 2 * NX:
                    make_v(g + 1)
                kblk = K[g][:, u * 128 : (u + 1) * 128]
                psS = psm.tile([C, NQ], F32, tag="psq", name=f"s{i}")
                for h in range(2):
                    sl = slice(h * 512, (h + 1) * 512)
                    nc.tensor.matmul(psS[:, sl], kblk, QT[:, sl], start=True, stop=True)
                if i > 0:
                    av(i - 1)
                E = ep.tile([C, NQ], F16, tag="e", name=f"e{i}")
                nc.scalar.activation(E, psS, Exp, bias=BT[:, i : i + 1])
                EL[i] = E
                # denominator: vector engine for blocks 0..27 (gpsimd would
                # steal the shared DVE SBUF port), PE ones-matmuls into PSUM
                # for the last 4 so no merge chain trails the loop
                if i < MB - 4:
                    if i == 0:
                        nc.vector.tensor_copy(ACCF, E)
                    else:
                        nc.vector.tensor_add(ACCF, ACCF, E)
                else:
                    if i == MB - 4:
                        PD[0] = psm.tile([1, 512], F32, tag="ps", name="pd0", bufs=2)
                        PD[1] = psm.tile([1, 512], F32, tag="ps", name="pd1", bufs=2)
                    for h in range(2):
                        sl = slice(h * 512, (h + 1) * 512)
                        nc.tensor.matmul(
                            PD[h], ONH, E[:, sl],
                            start=(i == MB - 4), stop=False,
                        )
            av(MB - 1)

            # ---- denominator row, 1/d, normalize, project, residual ----
            XSB = cst.tile([C, NQ], F32, tag="xsb")
            nc.vector.tensor_scalar_add(XSB, XS, FB)
            PDC = cst.tile([1, NQ], F32R, tag="pdc")
            PB = psm.tile([C, NQ], F32, tag="psq", name="pb")
            RB = cst.tile([C, NQ], F32, tag="rb")
            OUTN = cst.tile([C, NQ], F32R, tag="outn")
            PP = psm.tile([C, NQ], F32, tag="psq", name="pp")
            Y = cst.tile([C, NQ], F32, tag="y")
            for h in range(2):
                sl = slice(h * 512, (h + 1) * 512)
                nc.tensor.matmul(PD[h], ONC, ACCF[:, sl], start=False, stop=True)
                nc.scalar.activation(
                    PDC[:, sl], PD[h], mybir.ActivationFunctionType.Copy
                )
                nc.tensor.matmul(PB[:, sl], ONR, PDC[:, sl], start=True, stop=True)
                nc.vector.reciprocal_approx_fast(RB[:, sl], PB[:, sl])
                nc.vector.tensor_mul(OUTN[:, sl], PO[:, sl], RB[:, sl])
                nc.tensor.matmul(PP[:, sl], WPT, OUTN[:, sl], start=True, stop=True)
                nc.vector.tensor_add(Y[:, sl], PP[:, sl], XSB[:, sl])
                nc.sync.dma_start(y_d[:, sl], Y[:, sl])

    nc.compile()
    return nc


def _get_nc():
    if "nc" not in _CACHE:
        _CACHE["nc"] = _build()
    return _CACHE["nc"]


def kernel(
    x,
    gamma,
    beta,
    wq,
    bq,
    wk,
    bk,
    wv,
    bv,
    wp,
    bp,
    _results_hook=None,
    _run_kwargs=None,
    **_unused,
):
    from concourse.bass_utils import run_bass_kernel_spmd

    f = np.float32
    x = np.ascontiguousarray(np.asarray(x, dtype=f))
    Bx, Cx, D, Hh, W = x.shape
    NN = D * Hh * W
    xr = x.reshape(Bx, Cx, NN)

    gamma = np.asarray(gamma, f).reshape(C)
    beta = np.asarray(beta, f).reshape(C)
    wq = np.asarray(wq, f)
    wk = np.asarray(wk, f)
    wv = np.asarray(wv, f)
    wp = np.asarray(wp, f)
    bq = np.asarray(bq, f).reshape(C)
    bv = np.asarray(bv, f).reshape(C)
    bp = np.asarray(bp, f).reshape(C)

    scale = f(1.0) / np.sqrt(f(C))
    gsz = C // GROUPS

    per_batch = []
    for b in range(Bx):
        xg = xr[b].reshape(GROUPS, gsz * NN)
        mean_g = xg.mean(axis=1)
        var_g = xg.var(axis=1)
        s = (gamma.reshape(GROUPS, gsz) / np.sqrt(var_g + f(EPS))[:, None]).reshape(C)
        t = beta - np.repeat(mean_g, gsz) * s
        # fold the groupnorm affine into the weights: W' = W diag(s); b' = W t + b
        wqf = (wq * s[None, :]) * scale
        wkf = wk * s[None, :]
        wvf = wv * s[None, :]
        bqf = (wq @ t + bq) * scale
        bvf = wv @ t + bv
        fb = wp @ bvf + bp  # v-bias contribution + projection bias
        # score bias term (K^T bq'') folded into the exp bias, from raw x
        wstar = wkf.T @ bqf
        bterm = wstar @ xr[b]  # [N]
        wf_blob = np.concatenate([wqf.T, wkf.T, wvf.T], axis=1).astype(np.float16)
        fcols = np.concatenate(
            [fb[:, None], bterm.reshape(MB, C).T], axis=1
        ).astype(f)
        per_batch.append(
            {
                "xh": np.ascontiguousarray(xr[b]).astype(np.float16),
                "wf": np.ascontiguousarray(wf_blob),
                "fcols": np.ascontiguousarray(fcols),
            }
        )

    shared = {
        "wpt": np.ascontiguousarray(wp.T),
    }
    in_maps = []
    for core in range(8):
        b, sq = core // 4, core % 4
        xs = np.ascontiguousarray(xr[b][:, sq * NQ : (sq + 1) * NQ])
        in_maps.append(
            {
                **per_batch[b],
                "xsh": xs.astype(np.float16),
                "xs": xs,
                **shared,
            }
        )

    nc = _get_nc()
    res = None
    last_err = None
    for _attempt in range(3):
        try:
            res = run_bass_kernel_spmd(
                nc, in_maps, core_ids=list(range(8)), **(_run_kwargs or {})
            )
            break
        except Exception as e:  # transient NRT device errors: retry
            last_err = e
    if res is None:
        raise last_err
    if _results_hook is not None:
        _results_hook(res)

    out = np.empty((Bx, Cx, NN), f)
    for core in range(8):
        b, sq = core // 4, core % 4
        out[b][:, sq * NQ : (sq + 1) * NQ] = res.results[core]["y"]
    return out.reshape(Bx, Cx, D, Hh, W)



# revision 9
# speedup vs baseline: 1.0193x; 1.0193x over previous
"""BottleneckAttention3D kernel for 8 Trainium2 NeuronCores.

Reference computation (per batch b):
    h = GroupNorm(x)                      # [C, N], C=128, N=4096, 8 groups
    q = wq @ h + bq ; k = wk @ h + bk ; v = wv @ h + bv
    attn = softmax(q.T k / sqrt(C))       # [N, N]
    out = v attn.T ; y = x + wp @ out + bp
    (bk drops exactly: softmax is invariant to per-query shifts; the v bias
     reduces to a constant through the attn row-sum and folds into bp.)

Sharding: 8 cores = 2 batches x 4 query blocks of NQ=1024 tokens. Each core
runs a flash-attention-style loop over 32 key blocks of 128 tokens with the
score-block transpose layout [key, query]; the N^2 score matrix lives only in
PSUM/PSUM-sized tiles.

Host preprocessing: groupnorm statistics + affine fold into the QKV weights
(W' = W diag(s), b' = W t + b), fp16 casts, and the V projection (V^T shipped
pre-laid-out so the device AV matmul needs no per-block transposes).

Device-side structure per core (engine balance is the whole game; the Scalar
engine's 32 exp instructions are the ~35us floor, so everything else must
stay off ACT and under that budget):
  * ACT: exp only.  q-bias handled exactly by adding bq to Q's columns on DVE
    (K^T(Q+bq) == scores incl. bias term), so exp needs only the constant
    -2 bias (fp16-overflow headroom for the denominator sums).
  * PE: scores^T block = K-block^T Q (fp16, f32 PSUM, 3-deep ring) and
    attn*V accumulated in PSUM across the loop.
  * DVE: fp16 K-tile casts, and the softmax denominator as grouped fp16
    tree adds (2x mode) paced one per exp slot with a two-add tail.
  * Tail: ones-matmul collapse -> fp16 broadcast matmul ->
    reciprocal_approx_fast -> normalize -> fp16 projection -> residual.
"""

import sys

sys.path.insert(0, "/opt/trn_rl_repo")

import numpy as np

B = 2
C = 128
N = 4096  # 16*16*16 tokens
NQ = N // 4  # query block per core (1024)
GROUPS = 8
EPS = 1e-5
XCH = 1024
NX = N // XCH  # 4
MB = N // 128  # 32 key blocks
EBIAS = -2.0  # exp(s-2): scales num+denom equally, keeps fp16 sums < 1e4
_CACHE = {}


def _build():
    import concourse.bacc as bacc
    import concourse.mybir as mybir
    import concourse.tile as tile

    F32 = mybir.dt.float32
    F16 = mybir.dt.float16
    Exp = mybir.ActivationFunctionType.Exp
    Copy = mybir.ActivationFunctionType.Copy

    nc = bacc.Bacc("TRN2", target_bir_lowering=False, debug=False)

    # ---- DRAM I/O ----
    # wcat = [wq'.T | wk'.T | wp.T]  (fp16); fcol = [bq' | fb]  (f32)
    wcat_d = nc.dram_tensor("wcat", [C, 3 * C], F16, kind="ExternalInput")
    fcol_d = nc.dram_tensor("fcol", [C, 2], F32, kind="ExternalInput")
    xsh_d = nc.dram_tensor("xsh", [C, NQ], F16, kind="ExternalInput")
    xh_d = nc.dram_tensor("xh", [C, N], F16, kind="ExternalInput")
    vt_d = nc.dram_tensor("vt", [C, N], F16, kind="ExternalInput")
    y_d = nc.dram_tensor("y", [C, NQ], F32, kind="ExternalOutput")

    with tile.TileContext(nc) as tc:
        with (
            tc.tile_pool(name="cst", bufs=1) as cst,
            tc.tile_pool(name="xp", bufs=1) as xp,
            tc.tile_pool(name="ep", bufs=8) as ep,
            tc.tile_pool(name="psm", bufs=3, space="PSUM") as psm,
            tc.tile_pool(name="pso", bufs=1, space="PSUM") as pso,
        ):
            # dummy ACT op: load the exp table set at t=0
            DUM = cst.tile([1, 1], F32, tag="dum")
            nc.vector.memset(DUM, 1.0)
            DUM2 = cst.tile([1, 1], F32, tag="dum2")
            nc.scalar.activation(DUM2, DUM, Exp)

            # ---- input loads (sync queue: weights -> xsh -> xh; gpsimd: vt) ----
            WCAT = cst.tile([C, 3 * C], F16, tag="wcat")
            nc.sync.dma_start(WCAT, wcat_d[:, :])
            FCOL = cst.tile([C, 2], F32, tag="fcol")
            nc.sync.dma_start(FCOL, fcol_d[:, :])
            XSH = cst.tile([C, NQ], F16, tag="xsh")
            nc.sync.dma_start(XSH, xsh_d[:, :])
            XH = []
            for j in range(NX):
                xt = xp.tile([C, XCH], F16, tag=f"x{j}", name=f"x{j}")
                nc.sync.dma_start(xt, xh_d[:, j * XCH : (j + 1) * XCH])
                XH.append(xt)
            VT = cst.tile([C, N], F16, tag="vt")
            nc.gpsimd.dma_start(VT[:, : N // 2], vt_d[:, : N // 2])
            nc.gpsimd.dma_start(VT[:, N // 2 :], vt_d[:, N // 2 :])
            WQF = WCAT[:, 0 * C : 1 * C]
            WKF = WCAT[:, 1 * C : 2 * C]
            WPT = WCAT[:, 2 * C : 3 * C]
            BQ = FCOL[:, 0:1]
            FB = FCOL[:, 1:2]
            # ones vectors (fp16) and the constant exp bias column
            ONH = cst.tile([C, 1], F16, tag="onh")
            nc.vector.memset(ONH, 1.0)
            ONR = cst.tile([1, C], F16, tag="onr")
            nc.vector.memset(ONR, 1.0)
            EB = cst.tile([C, 1], F32, tag="eb")
            nc.vector.memset(EB, EBIAS)

            # ---- Q (with bias on DVE), residual base, K (casts on DVE) ----
            PQ = psm.tile([C, NQ], F32, tag="s", name="pq")
            for h in range(2):
                sl = slice(h * 512, (h + 1) * 512)
                nc.tensor.matmul(PQ[:, sl], WQF, XSH[:, sl], start=True, stop=True)
            QT = cst.tile([C, NQ], F16, tag="qt")
            nc.vector.tensor_scalar_add(QT, PQ, BQ)
            XSB = cst.tile([C, NQ], F16, tag="xsb")
            nc.vector.tensor_scalar_add(XSB, XSH, FB)

            K = [None] * (2 * NX)

            def make_k(j2):
                pk = psm.tile([C, 512], F32, tag="s", name=f"pk{j2}")
                nc.tensor.matmul(
                    pk,
                    WKF,
                    XH[j2 // 2][:, (j2 % 2) * 512 : (j2 % 2 + 1) * 512],
                    start=True,
                    stop=True,
                )
                kt = xp.tile([C, 512], F16, tag=f"k{j2}", name=f"k{j2}")
                nc.vector.tensor_copy(kt, pk)
                K[j2] = kt

            for j2 in range(4):
                make_k(j2)

            # ---- main attention loop ----
            PO = pso.tile([C, NQ], F32, tag="po")
            EL = [None] * MB
            # denominator tree state: 8 groups of 4 blocks, fp16
            G = [None] * 8
            RACC = [None]

            def av(i):
                for h in range(2):
                    sl = slice(h * 512, (h + 1) * 512)
                    nc.tensor.matmul(
                        PO[:, sl],
                        VT[:, i * 128 : (i + 1) * 128],
                        EL[i][:, sl],
                        start=(i == 0),
                        stop=(i == MB - 1),
                    )

            def dtree(i):
                # in-group left-deep adds; top chain merges groups 0..6 as
                # they complete; group 7 merges in the tail (short chain)
                g, u = i // 4, i % 4
                if u == 1:
                    t = ep.tile([C, NQ], F16, tag="g", name=f"g{g}", bufs=3)
                    nc.vector.tensor_add(t, EL[i - 1], EL[i])
                    G[g] = t
                elif u > 1:
                    nc.vector.tensor_add(G[g], G[g], EL[i])
                if u == 3 and g > 0:
                    if g == 1:
                        r = ep.tile([C, NQ], F16, tag="r", name="racc", bufs=1)
                        nc.vector.tensor_add(r, G[0], G[1])
                        RACC[0] = r
                    else:
                        nc.vector.tensor_add(RACC[0], RACC[0], G[g])

            for i in range(MB):
                if i % 4 == 1 and 4 + i // 4 < 2 * NX:
                    make_k(4 + i // 4)
                kblk = K[i // 4][:, (i % 4) * 128 : (i % 4 + 1) * 128]
                psS = psm.tile([C, NQ], F32, tag="s", name=f"s{i}")
                for h in range(2):
                    sl = slice(h * 512, (h + 1) * 512)
                    nc.tensor.matmul(psS[:, sl], kblk, QT[:, sl], start=True, stop=True)
                if i > 0:
                    av(i - 1)
                E = ep.tile([C, NQ], F16, tag="e", name=f"e{i}")
                nc.scalar.activation(E, psS, Exp, bias=EB)
                EL[i] = E
                dtree(i)
            av(MB - 1)
            ACC = RACC[0]
            nc.vector.tensor_add(ACC, ACC, G[7])  # only tail add after last exp

            # ---- denominator row, 1/d, normalize, project, residual ----
            for h in range(2):
                sl = slice(h * 512, (h + 1) * 512)
                PD = psm.tile([1, 512], F32, tag="s", name=f"pd{h}")
                nc.tensor.matmul(PD, ONH, ACC[:, sl], start=True, stop=True)
                PDC = cst.tile([1, 512], F16, tag=f"pdc{h}")
                nc.scalar.activation(PDC, PD, Copy)
                PB = psm.tile([C, 512], F32, tag="s", name=f"pb{h}")
                nc.tensor.matmul(PB, ONR, PDC, start=True, stop=True)
                RB = cst.tile([C, 512], F32, tag=f"rb{h}")
                nc.vector.reciprocal_approx_fast(RB, PB)
                OUTN = cst.tile([C, 512], F16, tag=f"outn{h}")
                nc.vector.tensor_mul(OUTN, PO[:, sl], RB)
                PP = psm.tile([C, 512], F32, tag="s", name=f"pp{h}")
                nc.tensor.matmul(PP, WPT, OUTN, start=True, stop=True)
                Y = cst.tile([C, 512], F32, tag=f"y{h}")
                nc.vector.tensor_add(Y, PP, XSB[:, sl])
                nc.sync.dma_start(y_d[:, sl], Y)

    nc.compile()
    return nc


def _get_nc():
    if "nc" not in _CACHE:
        _CACHE["nc"] = _build()
    return _CACHE["nc"]


def kernel(
    x,
    gamma,
    beta,
    wq,
    bq,
    wk,
    bk,
    wv,
    bv,
    wp,
    bp,
    _results_hook=None,
    _run_kwargs=None,
    **_unused,
):
    from concourse.bass_utils import run_bass_kernel_spmd

    f = np.float32
    x = np.ascontiguousarray(np.asarray(x, dtype=f))
    Bx, Cx, D, Hh, W = x.shape
    NN = D * Hh * W
    xr = x.reshape(Bx, Cx, NN)

    gamma = np.asarray(gamma, f).reshape(C)
    beta = np.asarray(beta, f).reshape(C)
    wq = np.asarray(wq, f)
    wk = np.asarray(wk, f)
    wv = np.asarray(wv, f)
    wp = np.asarray(wp, f)
    bq = np.asarray(bq, f).reshape(C)
    bv = np.asarray(bv, f).reshape(C)
    bp = np.asarray(bp, f).reshape(C)

    scale = f(1.0) / np.sqrt(f(C))
    gsz = C // GROUPS

    per_batch = []
    for b in range(Bx):
        xg = xr[b].reshape(GROUPS, gsz * NN)
        mean_g = xg.mean(axis=1)
        var_g = xg.var(axis=1)
        s = (gamma.reshape(GROUPS, gsz) / np.sqrt(var_g + f(EPS))[:, None]).reshape(C)
        t = beta - np.repeat(mean_g, gsz) * s
        # fold the groupnorm affine into the weights: W' = W diag(s); b' = W t + b
        wqf = (wq * s[None, :]) * scale
        wkf = wk * s[None, :]
        wvf = wv * s[None, :]
        bqf = (wq @ t + bq) * scale
        bvf = wv @ t + bv
        fb = wp @ bvf + bp  # v-bias contribution + projection bias
        # V^T computed on host, laid out as [p, blk*128 + c] = V[c, blk*128+p]
        vtb = (wvf @ xr[b]).reshape(C, MB, 128).transpose(2, 1, 0)
        wcat = np.concatenate([wqf.T, wkf.T, wp.T], axis=1).astype(np.float16)
        fcol = np.stack([bqf, fb], axis=1).astype(f)
        per_batch.append(
            {
                "xh": np.ascontiguousarray(xr[b]).astype(np.float16),
                "vt": np.ascontiguousarray(vtb.reshape(C, NN)).astype(np.float16),
                "wcat": np.ascontiguousarray(wcat),
                "fcol": np.ascontiguousarray(fcol),
            }
        )

    in_maps = []
    for core in range(8):
        b, sq = core // 4, core % 4
        xs = np.ascontiguousarray(xr[b][:, sq * NQ : (sq + 1) * NQ])
        in_maps.append(
            {
                **per_batch[b],
                "xsh": xs.astype(np.float16),
            }
        )

    nc = _get_nc()
    res = None
    last_err = None
    for _attempt in range(3):
        try:
            res = run_bass_kernel_spmd(
                nc, in_maps, core_ids=list(range(8)), **(_run_kwargs or {})
            )
            break
        except Exception as e:  # transient NRT device errors: retry
            last_err = e
    if res is None:
        raise last_err
    if _results_hook is not None:
        _results_hook(res)

    out = np.empty((Bx, Cx, NN), f)
    for core in range(8):
        b, sq = core // 4, core % 4
        out[b][:, sq * NQ : (sq + 1) * NQ] = res.results[core]["y"]
    return out.reshape(Bx, Cx, D, Hh, W)


# revision 16
# speedup vs baseline: 1.1063x; 1.0854x over previous
"""BottleneckAttention3D kernel for 8 Trainium2 NeuronCores.

Reference computation (per batch b):
    h = GroupNorm(x)                      # [C, N], C=128, N=4096, 8 groups
    q = wq @ h + bq ; k = wk @ h + bk ; v = wv @ h + bv
    attn = softmax(q.T k / sqrt(C))       # [N, N]
    out = v attn.T ; y = x + wp @ out + bp
    (bk drops exactly: softmax is invariant to per-query shifts; the v bias
     reduces to a constant through the attn row-sum and folds into bp; bq is
     added to Q's columns so the score bias needs no separate term.)

Sharding: 8 cores = 2 batches x 4 query blocks of NQ=1024 tokens. Each core
runs a flash-attention-style loop over 32 key blocks of 128 tokens in the
[key, query] score layout. Inputs are ROTATED per core so its own query
block is key-chunk 0 (attention is key-order invariant), which removes the
separate q-block load from the DMA critical path.

Host preprocessing: groupnorm statistics + affine fold into the QKV weights,
fp16 casts, and the V projection (V^T shipped pre-laid-out and pre-rotated).

Device-side engine balance (the Scalar engine's 32 exp instructions are the
~32us floor; everything else must stay off ACT and under that budget):
  * ACT: the exp stream + two early K-tile casts while it is otherwise idle.
  * PE: scores^T = K-block^T Q and attn*V accumulated in PSUM; warm-up
    matmuls into PO release the HAM clock gate before the loop starts.
  * DVE: Q bias add, K casts, and the denominator: in-group fp16 adds
    (2x mode) with an fp32 top chain (fp16 truncation bias otherwise costs
    ~1% on the row sums), one add per exp slot, two-add tail.
  * Tail: ones[128,128] matmul fuses the partition collapse AND broadcast
    of the denominator row; reciprocal_approx_fast -> normalize -> fp16
    projection -> residual; halves interleaved, y written fp16 on two
    DMA queues.
"""

import sys

sys.path.insert(0, "/opt/trn_rl_repo")

import numpy as np

B = 2
C = 128
N = 4096  # 16*16*16 tokens
NQ = N // 4  # query block per core (1024)
GROUPS = 8
EPS = 1e-5
KCH = 512
NK = N // KCH  # 8 K chunks
MB = N // 128  # 32 key blocks
EBIAS = -2.0  # exp(s-2): scales num+denom equally, keeps fp16 sums < 1e4
_CACHE = {}


def _build():
    import concourse.bacc as bacc
    import concourse.mybir as mybir
    import concourse.tile as tile

    F32 = mybir.dt.float32
    F16 = mybir.dt.float16
    Exp = mybir.ActivationFunctionType.Exp
    Copy = mybir.ActivationFunctionType.Copy

    nc = bacc.Bacc("TRN2", target_bir_lowering=False, debug=False)

    # ---- DRAM I/O ----
    wcat_d = nc.dram_tensor("wcat", [C, 2 * C], F16, kind="ExternalInput")
    wpt_d = nc.dram_tensor("wpt", [C, C], F16, kind="ExternalInput")
    fcol_d = nc.dram_tensor("fcol", [C, 2], F32, kind="ExternalInput")
    xh_d = nc.dram_tensor("xh", [C, N], F16, kind="ExternalInput")
    vt_d = nc.dram_tensor("vt", [C, N], F16, kind="ExternalInput")
    y_d = nc.dram_tensor("y", [C, NQ], F16, kind="ExternalOutput")

    with tile.TileContext(nc) as tc:
        with (
            tc.tile_pool(name="cst", bufs=1) as cst,
            tc.tile_pool(name="xp", bufs=1) as xp,
            tc.tile_pool(name="ep", bufs=8) as ep,
            tc.tile_pool(name="psm", bufs=3, space="PSUM") as psm,
            tc.tile_pool(name="pso", bufs=1, space="PSUM") as pso,
        ):
            # dummy ACT op: load the exp table set at t=0
            DUM = cst.tile([1, 1], F32, tag="dum")
            nc.vector.memset(DUM, 1.0)
            DUM2 = cst.tile([1, 1], F32, tag="dum2")
            nc.scalar.activation(DUM2, DUM, Exp)

            # constants (ONES doubles as the warm-up matmul operand)
            ONES = cst.tile([C, 512], F16, tag="ones")
            nc.vector.memset(ONES, 1.0)
            ONES32 = cst.tile([C, C], F32, tag="ones32")
            nc.vector.memset(ONES32, 1.0)
            EB = cst.tile([C, 1], F32, tag="eb")
            nc.vector.memset(EB, EBIAS)

            # ---- input loads ----
            # sync queue: weights -> xh chunks (critical path); vt1/vt3 after.
            # gpsimd queue: fcol, vt0, vt2, wpt.
            WCAT = cst.tile([C, 2 * C], F16, tag="wcat")
            nc.sync.dma_start(WCAT, wcat_d[:, :])
            XH = []
            for j in range(NK):
                xt = xp.tile([C, KCH], F16, tag=f"x{j}", name=f"x{j}")
                nc.sync.dma_start(xt, xh_d[:, j * KCH : (j + 1) * KCH])
                XH.append(xt)
            FCOL = cst.tile([C, 2], F32, tag="fcol")
            nc.gpsimd.dma_start(FCOL, fcol_d[:, :])
            VT = cst.tile([C, N], F16, tag="vt")
            nc.gpsimd.dma_start(VT[:, 0:1024], vt_d[:, 0:1024])
            nc.sync.dma_start(VT[:, 1024:2048], vt_d[:, 1024:2048])
            nc.gpsimd.dma_start(VT[:, 2048:3072], vt_d[:, 2048:3072])
            nc.sync.dma_start(VT[:, 3072:4096], vt_d[:, 3072:4096])
            WPT = cst.tile([C, C], F16, tag="wpt")
            nc.gpsimd.dma_start(WPT, wpt_d[:, :])
            WQF = WCAT[:, 0:C]
            WKF = WCAT[:, C : 2 * C]
            BQ = FCOL[:, 0:1]
            FB = FCOL[:, 1:2]

            # ---- PE warm-up: release the HAM clock gate before the loop ----
            PO = pso.tile([C, NQ], F32, tag="po")
            for w in range(8):
                nc.tensor.matmul(
                    PO[:, 0:512], ONES[:, 0:C], ONES, start=True, stop=True
                )

            # ---- Q (bias on DVE, halves), K tiles ----
            PQ = psm.tile([C, NQ], F32, tag="s", name="pq")
            QT = cst.tile([C, NQ], F16, tag="qt")
            for h in range(2):
                sl = slice(h * 512, (h + 1) * 512)
                nc.tensor.matmul(
                    PQ[:, sl], WQF, XH[h][:, 0:512], start=True, stop=True
                )
                nc.vector.tensor_scalar_add(QT[:, sl], PQ[:, sl], BQ)

            K = [None] * NK

            def make_k(j, eng):
                pk = psm.tile([C, KCH], F32, tag="s", name=f"pk{j}")
                nc.tensor.matmul(pk, WKF, XH[j], start=True, stop=True)
                kt = xp.tile([C, KCH], F16, tag=f"k{j}", name=f"k{j}")
                if eng == "act":
                    nc.scalar.activation(kt, pk, Copy)
                else:
                    nc.vector.tensor_copy(kt, pk)
                K[j] = kt

            make_k(0, "act")
            make_k(1, "act")

            # ---- main attention loop ----
            EL = [None] * MB
            G = [None] * 8
            RACC = [None]

            def av(i):
                for h in range(2):
                    sl = slice(h * 512, (h + 1) * 512)
                    nc.tensor.matmul(
                        PO[:, sl],
                        VT[:, i * 128 : (i + 1) * 128],
                        EL[i][:, sl],
                        start=(i == 0),
                        stop=(i == MB - 1),
                    )

            def dtree(i):
                # in-group (4 blocks) left-deep fp16 adds; fp32 top chain
                # merges groups 0..6 in-loop; group 7 merges in the tail
                g, u = i // 4, i % 4
                if u == 1:
                    t = ep.tile([C, NQ], F16, tag="g", name=f"g{g}", bufs=3)
                    nc.vector.tensor_add(t, EL[i - 1], EL[i])
                    G[g] = t
                elif u > 1:
                    nc.vector.tensor_add(G[g], G[g], EL[i])
                if u == 3 and g > 0:
                    if g == 1:
                        r = ep.tile([C, NQ], F32, tag="r", name="racc", bufs=1)
                        nc.vector.tensor_add(r, G[0], G[1])
                        RACC[0] = r
                    else:
                        nc.vector.tensor_add(RACC[0], RACC[0], G[g])

            for i in range(MB):
                if i % 2 == 1 and 2 + i // 2 < NK:
                    make_k(2 + i // 2, "dve")
                kblk = K[i // 4][:, (i % 4) * 128 : (i % 4 + 1) * 128]
                psS = psm.tile([C, NQ], F32, tag="s", name=f"s{i}")
                for h in range(2):
                    sl = slice(h * 512, (h + 1) * 512)
                    nc.tensor.matmul(psS[:, sl], kblk, QT[:, sl], start=True, stop=True)
                if i > 0:
                    av(i - 1)
                E = ep.tile([C, NQ], F16, tag="e", name=f"e{i}")
                nc.scalar.activation(E, psS, Exp, bias=EB)
                EL[i] = E
                dtree(i)
            av(MB - 1)
            ACC = RACC[0]
            nc.vector.tensor_add(ACC, ACC, G[7])  # only tail add after last exp

            # ---- residual base (needed only in the tail) ----
            XSB = cst.tile([C, NQ], F16, tag="xsb")
            for h in range(2):
                sl = slice(h * 512, (h + 1) * 512)
                nc.vector.tensor_scalar_add(XSB[:, sl], XH[h], FB)

            # ---- denominator bcast, 1/d, normalize, project, residual ----
            # ones[128,128] @ ACC fuses the partition collapse and the
            # broadcast of the denominator row in a single matmul.
            PBs, RBs, OUTNs, PPs = [], [], [], []
            for h in range(2):
                sl = slice(h * 512, (h + 1) * 512)
                PB = psm.tile([C, 512], F32, tag="s", name=f"pb{h}")
                nc.tensor.matmul(PB, ONES32, ACC[:, sl], start=True, stop=True)
                PBs.append(PB)
            for h in range(2):
                RB = cst.tile([C, 512], F32, tag=f"rb{h}")
                nc.vector.reciprocal_approx_fast(RB, PBs[h])
                RBs.append(RB)
            for h in range(2):
                sl = slice(h * 512, (h + 1) * 512)
                OUTN = cst.tile([C, 512], F16, tag=f"outn{h}")
                nc.vector.tensor_mul(OUTN, PO[:, sl], RBs[h])
                OUTNs.append(OUTN)
            for h in range(2):
                PP = psm.tile([C, 512], F32, tag="s", name=f"pp{h}")
                nc.tensor.matmul(PP, WPT, OUTNs[h], start=True, stop=True)
                PPs.append(PP)
            for h in range(2):
                sl = slice(h * 512, (h + 1) * 512)
                Y = cst.tile([C, 512], F16, tag=f"y{h}")
                nc.vector.tensor_add(Y, PPs[h], XSB[:, sl])
                if h == 0:
                    nc.gpsimd.dma_start(y_d[:, sl], Y)
                else:
                    nc.sync.dma_start(y_d[:, sl], Y)

    nc.compile()
    return nc


def _get_nc():
    if "nc" not in _CACHE:
        _CACHE["nc"] = _build()
    return _CACHE["nc"]


def kernel(
    x,
    gamma,
    beta,
    wq,
    bq,
    wk,
    bk,
    wv,
    bv,
    wp,
    bp,
    _results_hook=None,
    _run_kwargs=None,
    **_unused,
):
    from concourse.bass_utils import run_bass_kernel_spmd

    f = np.float32
    x = np.ascontiguousarray(np.asarray(x, dtype=f))
    Bx, Cx, D, Hh, W = x.shape
    NN = D * Hh * W
    xr = x.reshape(Bx, Cx, NN)

    gamma = np.asarray(gamma, f).reshape(C)
    beta = np.asarray(beta, f).reshape(C)
    wq = np.asarray(wq, f)
    wk = np.asarray(wk, f)
    wv = np.asarray(wv, f)
    wp = np.asarray(wp, f)
    bq = np.asarray(bq, f).reshape(C)
    bv = np.asarray(bv, f).reshape(C)
    bp = np.asarray(bp, f).reshape(C)

    scale = f(1.0) / np.sqrt(f(C))
    gsz = C // GROUPS

    per_batch = []
    for b in range(Bx):
        xg = xr[b].reshape(GROUPS, gsz * NN)
        mean_g = xg.mean(axis=1)
        var_g = xg.var(axis=1)
        s = (gamma.reshape(GROUPS, gsz) / np.sqrt(var_g + f(EPS))[:, None]).reshape(C)
        t = beta - np.repeat(mean_g, gsz) * s
        # fold the groupnorm affine into the weights: W' = W diag(s); b' = W t + b
        wqf = (wq * s[None, :]) * scale
        wkf = wk * s[None, :]
        wvf = wv * s[None, :]
        bqf = (wq @ t + bq) * scale
        bvf = wv @ t + bv
        fb = wp @ bvf + bp  # v-bias contribution + projection bias
        # V^T on host, tile-layout [p, blk*128 + c] = V[c, blk*128 + p]
        vtb = (wvf @ xr[b]).reshape(C, MB, 128).transpose(2, 1, 0)
        wcat = np.concatenate([wqf.T, wkf.T], axis=1).astype(np.float16)
        fcol = np.stack([bqf, fb], axis=1).astype(f)
        per_batch.append(
            {
                "xh16": xr[b].astype(np.float16),
                "vtb": vtb.astype(np.float16),
                "wcat": np.ascontiguousarray(wcat),
                "wpt": np.ascontiguousarray(wp.T).astype(np.float16),
                "fcol": np.ascontiguousarray(fcol),
            }
        )

    in_maps = []
    for core in range(8):
        b, sq = core // 4, core % 4
        pb = per_batch[b]
        # rotate keys so this core's query block is chunk 0
        r = sq * NQ
        xh = np.concatenate([pb["xh16"][:, r:], pb["xh16"][:, :r]], axis=1)
        rb = sq * (NQ // 128)
        vtr = np.concatenate([pb["vtb"][:, rb:, :], pb["vtb"][:, :rb, :]], axis=1)
        in_maps.append(
            {
                "xh": np.ascontiguousarray(xh),
                "vt": np.ascontiguousarray(vtr.reshape(C, NN)),
                "wcat": pb["wcat"],
                "wpt": pb["wpt"],
                "fcol": pb["fcol"],
            }
        )

    nc = _get_nc()
    res = None
    last_err = None
    for _attempt in range(3):
        try:
            res = run_bass_kernel_spmd(
                nc, in_maps, core_ids=list(range(8)), **(_run_kwargs or {})
            )
            break
        except Exception as e:  # transient NRT device errors: retry
            last_err = e
    if res is None:
        raise last_err
    if _results_hook is not None:
        _results_hook(res)

    out = np.empty((Bx, Cx, NN), f)
    for core in range(8):
        b, sq = core // 4, core % 4
        out[b][:, sq * NQ : (sq + 1) * NQ] = res.results[core]["y"].astype(f)
    return out.reshape(Bx, Cx, D, Hh, W)


# revision 17
# speedup vs baseline: 1.1271x; 1.0188x over previous
"""BottleneckAttention3D kernel for 8 Trainium2 NeuronCores.

Reference computation (per batch b):
    h = GroupNorm(x)                      # [C, N], C=128, N=4096, 8 groups
    q = wq @ h + bq ; k = wk @ h + bk ; v = wv @ h + bv
    attn = softmax(q.T k / sqrt(C))       # [N, N]
    out = v attn.T ; y = x + wp @ out + bp
    (bk drops exactly: softmax is invariant to per-query shifts; the v bias
     reduces to a constant through the attn row-sum and folds into bp; bq is
     added to Q's columns so the score bias needs no separate term.)

Sharding: 8 cores = 2 batches x 4 query blocks of NQ=1024 tokens. Each core
runs a flash-attention-style loop over 32 key blocks of 128 tokens in the
[key, query] score layout. Inputs are ROTATED per core so its own query
block is key-chunk 0 (attention is key-order invariant), which removes the
separate q-block load from the DMA critical path.

Host preprocessing: groupnorm statistics + affine fold into the QKV weights,
fp16 casts, and the V projection (V^T shipped pre-laid-out and pre-rotated).

Device-side engine balance (the Scalar engine's 32 exp instructions are the
~32us floor; everything else must stay off ACT and under that budget):
  * ACT: the exp stream + two early K-tile casts while it is otherwise idle.
  * PE: scores^T = K-block^T Q and attn*V accumulated in PSUM; warm-up
    matmuls into PO release the HAM clock gate before the loop starts.
  * DVE: Q bias add, K casts, and the denominator: in-group fp16 adds
    (2x mode) with an fp32 top chain (fp16 truncation bias otherwise costs
    ~1% on the row sums), one add per exp slot, two-add tail.
  * Tail: ones[128,128] matmul fuses the partition collapse AND broadcast
    of the denominator row; reciprocal_approx_fast -> normalize -> fp16
    projection -> residual; halves interleaved, y written fp16 on two
    DMA queues.
"""

import sys

sys.path.insert(0, "/opt/trn_rl_repo")

import numpy as np

B = 2
C = 128
N = 4096  # 16*16*16 tokens
NQ = N // 4  # query block per core (1024)
GROUPS = 8
EPS = 1e-5
KCH = 512
NK = N // KCH  # 8 K chunks
MB = N // 128  # 32 key blocks
EBIAS = -2.0  # exp(s-2): scales num+denom equally, keeps fp16 sums < 1e4
_CACHE = {}


def _build():
    import concourse.bacc as bacc
    import concourse.mybir as mybir
    import concourse.tile as tile

    F32 = mybir.dt.float32
    F16 = mybir.dt.float16
    Exp = mybir.ActivationFunctionType.Exp
    Copy = mybir.ActivationFunctionType.Copy

    nc = bacc.Bacc("TRN2", target_bir_lowering=False, debug=False)

    # ---- DRAM I/O ----
    wcat_d = nc.dram_tensor("wcat", [C, 2 * C], F16, kind="ExternalInput")
    wpt_d = nc.dram_tensor("wpt", [C, C], F16, kind="ExternalInput")
    fcol_d = nc.dram_tensor("fcol", [C, 2], F32, kind="ExternalInput")
    xh_d = nc.dram_tensor("xh", [C, N], F16, kind="ExternalInput")
    vt_d = nc.dram_tensor("vt", [C, N], F16, kind="ExternalInput")
    y_d = nc.dram_tensor("y", [C, NQ], F16, kind="ExternalOutput")

    with tile.TileContext(nc) as tc:
        with (
            tc.tile_pool(name="cst", bufs=1) as cst,
            tc.tile_pool(name="xp", bufs=1) as xp,
            tc.tile_pool(name="ep", bufs=8) as ep,
            tc.tile_pool(name="psm", bufs=3, space="PSUM") as psm,
            tc.tile_pool(name="pso", bufs=1, space="PSUM") as pso,
        ):
            # dummy ACT op: load the exp table set at t=0
            DUM = cst.tile([1, 1], F32, tag="dum")
            nc.vector.memset(DUM, 1.0)
            DUM2 = cst.tile([1, 1], F32, tag="dum2")
            nc.scalar.activation(DUM2, DUM, Exp)

            # constants (ONES doubles as the warm-up matmul operand)
            ONES = cst.tile([C, 512], F16, tag="ones")
            nc.vector.memset(ONES, 1.0)
            ONES32 = cst.tile([C, C], F32, tag="ones32")
            nc.vector.memset(ONES32, 1.0)
            EB = cst.tile([C, 1], F32, tag="eb")
            nc.vector.memset(EB, EBIAS)

            # ---- input loads ----
            # sync queue: weights -> xh chunks (critical path); vt1/vt3 after.
            # gpsimd queue: fcol, vt0, vt2, wpt.
            WCAT = cst.tile([C, 2 * C], F16, tag="wcat")
            nc.sync.dma_start(WCAT, wcat_d[:, :])
            XH = []
            for j in range(NK):
                xt = xp.tile([C, KCH], F16, tag=f"x{j}", name=f"x{j}")
                nc.sync.dma_start(xt, xh_d[:, j * KCH : (j + 1) * KCH])
                XH.append(xt)
            FCOL = cst.tile([C, 2], F32, tag="fcol")
            nc.gpsimd.dma_start(FCOL, fcol_d[:, :])
            VT = cst.tile([C, N], F16, tag="vt")
            nc.gpsimd.dma_start(VT[:, 0:1024], vt_d[:, 0:1024])
            nc.sync.dma_start(VT[:, 1024:2048], vt_d[:, 1024:2048])
            nc.gpsimd.dma_start(VT[:, 2048:3072], vt_d[:, 2048:3072])
            nc.sync.dma_start(VT[:, 3072:4096], vt_d[:, 3072:4096])
            WPT = cst.tile([C, C], F16, tag="wpt")
            nc.gpsimd.dma_start(WPT, wpt_d[:, :])
            WQF = WCAT[:, 0:C]
            WKF = WCAT[:, C : 2 * C]
            BQ = FCOL[:, 0:1]
            FB = FCOL[:, 1:2]

            # ---- PE warm-up: release the HAM clock gate before the loop ----
            PO = pso.tile([C, NQ], F32, tag="po")
            for w in range(8):
                nc.tensor.matmul(
                    PO[:, 0:512], ONES[:, 0:C], ONES, start=True, stop=True
                )

            # ---- Q (bias on DVE, halves), K tiles ----
            PQ = psm.tile([C, NQ], F32, tag="s", name="pq")
            QT = cst.tile([C, NQ], F16, tag="qt")
            for h in range(2):
                sl = slice(h * 512, (h + 1) * 512)
                nc.tensor.matmul(
                    PQ[:, sl], WQF, XH[h][:, 0:512], start=True, stop=True
                )
                nc.vector.tensor_scalar_add(QT[:, sl], PQ[:, sl], BQ)

            K = [None] * NK

            def make_k(j, eng):
                pk = psm.tile([C, KCH], F32, tag="s", name=f"pk{j}")
                nc.tensor.matmul(pk, WKF, XH[j], start=True, stop=True)
                kt = xp.tile([C, KCH], F16, tag=f"k{j}", name=f"k{j}")
                if eng == "act":
                    nc.scalar.activation(kt, pk, Copy)
                else:
                    nc.vector.tensor_copy(kt, pk)
                K[j] = kt

            make_k(0, "act")
            make_k(1, "act")

            # ---- main attention loop ----
            EL = [None] * MB
            G = [None] * 8
            RACC = [None]

            def av(i):
                for h in range(2):
                    sl = slice(h * 512, (h + 1) * 512)
                    nc.tensor.matmul(
                        PO[:, sl],
                        VT[:, i * 128 : (i + 1) * 128],
                        EL[i][:, sl],
                        start=(i == 0),
                        stop=(i == MB - 1),
                    )

            def dtree(i):
                # in-group (4 blocks) left-deep fp16 adds; fp32 top chain
                # merges groups 0..6 in-loop; group 7 merges in the tail
                g, u = i // 4, i % 4
                if u == 1:
                    t = ep.tile([C, NQ], F16, tag="g", name=f"g{g}", bufs=3)
                    nc.vector.tensor_add(t, EL[i - 1], EL[i])
                    G[g] = t
                elif u > 1:
                    nc.vector.tensor_add(G[g], G[g], EL[i])
                if u == 3 and 0 < g < 7:  # g7 merges post-loop (short tail)
                    if g == 1:
                        r = ep.tile([C, NQ], F32, tag="r", name="racc", bufs=1)
                        nc.vector.tensor_add(r, G[0], G[1])
                        RACC[0] = r
                    else:
                        nc.vector.tensor_add(RACC[0], RACC[0], G[g])

            for i in range(MB):
                if i % 2 == 1 and 2 + i // 2 < NK:
                    make_k(2 + i // 2, "dve")
                kblk = K[i // 4][:, (i % 4) * 128 : (i % 4 + 1) * 128]
                psS = psm.tile([C, NQ], F32, tag="s", name=f"s{i}")
                for h in range(2):
                    sl = slice(h * 512, (h + 1) * 512)
                    nc.tensor.matmul(psS[:, sl], kblk, QT[:, sl], start=True, stop=True)
                if i > 0:
                    av(i - 1)
                E = ep.tile([C, NQ], F16, tag="e", name=f"e{i}")
                nc.scalar.activation(E, psS, Exp, bias=EB)
                EL[i] = E
                dtree(i)
            av(MB - 1)
            ACC = RACC[0]
            nc.vector.tensor_add(ACC, ACC, G[7])  # only tail add after last exp

            # ---- residual base (needed only in the tail) ----
            XSB = cst.tile([C, NQ], F16, tag="xsb")
            for h in range(2):
                sl = slice(h * 512, (h + 1) * 512)
                nc.vector.tensor_scalar_add(XSB[:, sl], XH[h], FB)

            # ---- denominator bcast, 1/d, normalize, project, residual ----
            # ones[128,128] @ ACC fuses the partition collapse and the
            # broadcast of the denominator row in a single matmul.
            PBs, RBs, OUTNs, PPs = [], [], [], []
            for h in range(2):
                sl = slice(h * 512, (h + 1) * 512)
                PB = psm.tile([C, 512], F32, tag="s", name=f"pb{h}")
                nc.tensor.matmul(PB, ONES32, ACC[:, sl], start=True, stop=True)
                PBs.append(PB)
            for h in range(2):
                RB = cst.tile([C, 512], F32, tag=f"rb{h}")
                nc.vector.reciprocal_approx_fast(RB, PBs[h])
                RBs.append(RB)
            for h in range(2):
                sl = slice(h * 512, (h + 1) * 512)
                OUTN = cst.tile([C, 512], F16, tag=f"outn{h}")
                nc.vector.tensor_mul(OUTN, PO[:, sl], RBs[h])
                OUTNs.append(OUTN)
            for h in range(2):
                PP = psm.tile([C, 512], F32, tag="s", name=f"pp{h}")
                nc.tensor.matmul(PP, WPT, OUTNs[h], start=True, stop=True)
                PPs.append(PP)
            for h in range(2):
                sl = slice(h * 512, (h + 1) * 512)
                Y = cst.tile([C, 512], F16, tag=f"y{h}")
                nc.vector.tensor_add(Y, PPs[h], XSB[:, sl])
                if h == 0:
                    nc.gpsimd.dma_start(y_d[:, sl], Y)
                else:
                    nc.sync.dma_start(y_d[:, sl], Y)

    nc.compile()
    return nc


def _get_nc():
    if "nc" not in _CACHE:
        _CACHE["nc"] = _build()
    return _CACHE["nc"]


def kernel(
    x,
    gamma,
    beta,
    wq,
    bq,
    wk,
    bk,
    wv,
    bv,
    wp,
    bp,
    _results_hook=None,
    _run_kwargs=None,
    **_unused,
):
    from concourse.bass_utils import run_bass_kernel_spmd

    f = np.float32
    x = np.ascontiguousarray(np.asarray(x, dtype=f))
    Bx, Cx, D, Hh, W = x.shape
    NN = D * Hh * W
    xr = x.reshape(Bx, Cx, NN)

    gamma = np.asarray(gamma, f).reshape(C)
    beta = np.asarray(beta, f).reshape(C)
    wq = np.asarray(wq, f)
    wk = np.asarray(wk, f)
    wv = np.asarray(wv, f)
    wp = np.asarray(wp, f)
    bq = np.asarray(bq, f).reshape(C)
    bv = np.asarray(bv, f).reshape(C)
    bp = np.asarray(bp, f).reshape(C)

    scale = f(1.0) / np.sqrt(f(C))
    gsz = C // GROUPS

    per_batch = []
    for b in range(Bx):
        xg = xr[b].reshape(GROUPS, gsz * NN)
        mean_g = xg.mean(axis=1)
        var_g = xg.var(axis=1)
        s = (gamma.reshape(GROUPS, gsz) / np.sqrt(var_g + f(EPS))[:, None]).reshape(C)
        t = beta - np.repeat(mean_g, gsz) * s
        # fold the groupnorm affine into the weights: W' = W diag(s); b' = W t + b
        wqf = (wq * s[None, :]) * scale
        wkf = wk * s[None, :]
        wvf = wv * s[None, :]
        bqf = (wq @ t + bq) * scale
        bvf = wv @ t + bv
        fb = wp @ bvf + bp  # v-bias contribution + projection bias
        # V^T on host, tile-layout [p, blk*128 + c] = V[c, blk*128 + p]
        vtb = (wvf @ xr[b]).reshape(C, MB, 128).transpose(2, 1, 0)
        wcat = np.concatenate([wqf.T, wkf.T], axis=1).astype(np.float16)
        fcol = np.stack([bqf, fb], axis=1).astype(f)
        per_batch.append(
            {
                "xh16": xr[b].astype(np.float16),
                "vtb": vtb.astype(np.float16),
                "wcat": np.ascontiguousarray(wcat),
                "wpt": np.ascontiguousarray(wp.T).astype(np.float16),
                "fcol": np.ascontiguousarray(fcol),
            }
        )

    in_maps = []
    for core in range(8):
        b, sq = core // 4, core % 4
        pb = per_batch[b]
        # rotate keys so this core's query block is chunk 0
        r = sq * NQ
        xh = np.concatenate([pb["xh16"][:, r:], pb["xh16"][:, :r]], axis=1)
        rb = sq * (NQ // 128)
        vtr = np.concatenate([pb["vtb"][:, rb:, :], pb["vtb"][:, :rb, :]], axis=1)
        in_maps.append(
            {
                "xh": np.ascontiguousarray(xh),
                "vt": np.ascontiguousarray(vtr.reshape(C, NN)),
                "wcat": pb["wcat"],
                "wpt": pb["wpt"],
                "fcol": pb["fcol"],
            }
        )

    nc = _get_nc()
    res = None
    last_err = None
    for _attempt in range(3):
        try:
            res = run_bass_kernel_spmd(
                nc, in_maps, core_ids=list(range(8)), **(_run_kwargs or {})
            )
            break
        except Exception as e:  # transient NRT device errors: retry
            last_err = e
    if res is None:
        raise last_err
    if _results_hook is not None:
        _results_hook(res)

    out = np.empty((Bx, Cx, NN), f)
    for core in range(8):
        b, sq = core // 4, core % 4
        out[b][:, sq * NQ : (sq + 1) * NQ] = res.results[core]["y"].astype(f)
    return out.reshape(Bx, Cx, D, Hh, W)
